# revision 26
# baseline (speedup 1.0000x reference)
"""3-layer GAT (GATConv x3 + log_softmax) on 8 trn2 NeuronCores — v3.

Strategy (dst-node 1-D partition, slot-major exchange):
- Edges on the dst-owner core, sorted by dst, packed in windows of <=128 dst
  nodes / <=K_TILES*128 edges. Per-window DRAM addressing is slot-major
  (window w <-> rows [w*128,(w+1)*128)), identical on every core, so all
  window writes/reads are plain DMAs; the slot->node reorder is host numpy.
- Layer 1 aggregates in x-space: since sum_e alpha_e * (x W1)[s_e] =
  (sum_e alpha_e x[s_e]) W1 per head, the per-edge gather is 56B (x|als|ald
  bf16) instead of 1KB, there is no materialized h1 table, and the per-tile
  aggregation matmul streams 96 cols instead of 512.
- Per-edge source rows come via per-tile indirect DMA ([128,1] offsets is
  the only layout the INDIRECT1D ucode supports). al_d comes per *window*
  (<=128 dst nodes): layer 1 via one tiny indirect gather, layers 2/3 via a
  plain slot-major load; it is expanded per edge with a one-hot matmul
  against a host-prebuilt selT matrix streamed from DRAM (selcat).
- exp(lrelu(x)) = max(exp(x), exp(0.2x)) keeps the scalar engine pinned to
  the Exp table; layer-3 log_softmax is batched over all windows at the end
  (one Exp + one Ln total).
- Segment softmax + scatter via one-hot sel matmuls accumulating in PSUM;
  next-layer features are produced per-window (fused projection) using an
  xbar DMA transpose; h2/h3 exchanges are single AllGathers.
"""
import numpy as np
import ml_dtypes

import concourse.bass as bass
import concourse.mybir as mybir
import concourse.tile as tile
from concourse.bass_utils import run_bass_kernel_spmd

BF = ml_dtypes.bfloat16
N = 50000
NC = 8
SHARD = N // NC            # 6250
H, C = 8, 64
F = H * C                  # 512
C3 = 5
F3 = H * C3                # 40
FX = 12                    # input feature width
XR = 28                    # X1 row: x(12) | als1(8) | ald1(8)
K_TILES = 8
G = 4
NEG_SLOPE = 0.2
DT_BF = mybir.dt.bfloat16
DT_F32 = mybir.dt.float32
DT_I32 = mybir.dt.int32
AF = mybir.ActivationFunctionType
ALU = mybir.AluOpType


def _split_drain_waits(nc, max_waits=1):
    # walrus on this toolchain rejects instructions carrying more than a few
    # sync waits; keep <=max_waits per instruction, move extras onto NoOps
    # inserted just before (same engine -> executes first, semantics kept).
    ctr = 0
    for f in nc.m.functions:
        for blk in f.blocks:
            new_list = []
            for ins in blk.instructions:
                if ins.sync_info and \
                        len(ins.sync_info.on_wait) > max_waits:
                    waits = list(ins.sync_info.on_wait)
                    keep, extra = waits[:max_waits], waits[max_waits:]
                    for w in extra:
                        ctr += 1
                        new_list.append(mybir.InstNoOp(
                            name=f"drainfix-{ctr}", engine=ins.engine,
                            ins=[], outs=[],
                            sync_info=mybir.SyncInfo(on_wait=[w], on_update=[])))
                    ins.sync_info.on_wait = keep
                new_list.append(ins)
            blk.instructions[:] = new_list


def _v(ap, dims):
    """AP over ap's tensor/offset with explicit free [step, count] dims."""
    return bass.AP(ap.tensor, ap.offset, [ap.ap[0]] + dims)


def host_prep(edge_index):
    src = np.concatenate([edge_index[0], np.arange(N, dtype=np.int32)]).astype(np.int64)
    dst = np.concatenate([edge_index[1], np.arange(N, dtype=np.int32)]).astype(np.int64)
    order = np.argsort(dst, kind="stable")
    src, dst = src[order], dst[order]
    cap = K_TILES * 128
    cores = []
    for c in range(NC):
        lo, hi = c * SHARD, (c + 1) * SHARD
        m0 = np.searchsorted(dst, lo, "left")
        m1 = np.searchsorted(dst, hi, "left")
        s_c, d_c = src[m0:m1], dst[m0:m1] - lo
        counts = np.bincount(d_c, minlength=SHARD)
        starts = np.concatenate([[0], np.cumsum(counts)])
        wins = []
        n0 = 0
        while n0 < SHARD:
            n1 = n0
            while n1 < SHARD and (n1 - n0) < 128 and \
                    (starts[n1 + 1] - starts[n0]) <= cap:
                n1 += 1
            if n1 == n0:
                n1 = n0 + 1
            wins.append((n0, n1))
            n0 = n1
        cores.append((s_c, d_c, starts, wins))
    W = max(len(c[3]) for c in cores)
    T = W * K_TILES
    WS = W * 128

    slot_of = np.zeros((NC, SHARD), np.int64)
    for c, (_, _, _, wins) in enumerate(cores):
        for w, (n0, n1) in enumerate(wins):
            slot_of[c, n0:n1] = w * 128 + np.arange(n1 - n0)

    def h2row(s):                        # global node -> row in Hf2 [NC*WS]
        return (s // SHARD) * WS + slot_of[s // SHARD, s % SHARD]

    es1 = np.zeros((NC, T, 128), np.int32)
    es2 = np.zeros((NC, T, 128), np.int32)
    wng = np.zeros((NC, W, 128), np.int32)       # window nodes (global id)
    # selcat: per window 2048 cols: 8 sel tiles [e,d] then 8 selT tiles [d,e]
    selcat = np.zeros((NC, 128, T * 256), BF)
    for c, (s_c, d_c, starts, wins) in enumerate(cores):
        for w, (n0, n1) in enumerate(wins):
            e0, e1 = starts[n0], starts[n1]
            ne = e1 - e0
            t0 = w * K_TILES
            ss = s_c[e0:e1]
            es1[c, t0:t0 + K_TILES].reshape(-1)[:ne] = ss
            es2[c, t0:t0 + K_TILES].reshape(-1)[:ne] = h2row(ss)
            wng[c, w, :n1 - n0] = np.arange(n0, n1) + c * SHARD
            dr = np.full(K_TILES * 128, 999, np.int64)
            dr[:ne] = d_c[e0:e1] - n0
            base = w * 2048
            for j in range(K_TILES):
                drj = dr[j * 128:(j + 1) * 128]
                m = drj < 128
                e_pos = np.nonzero(m)[0]
                d_pos = drj[m]
                blk = np.zeros((128, 128), BF)
                blk[e_pos, d_pos] = 1.0
                selcat[c, :, base + j * 128: base + (j + 1) * 128] = blk
                selcat[c, :, base + 1024 + j * 128: base + 1024 + (j + 1) * 128] = blk.T
    return (np.ascontiguousarray(es1.transpose(0, 2, 1)),
            np.ascontiguousarray(es2.transpose(0, 2, 1)),
            np.ascontiguousarray(wng.transpose(0, 2, 1)),
            selcat, W, T, slot_of)


def blockdiag(a):
    Hh, cc = a.shape
    out = np.zeros((Hh * cc, Hh), np.float32)
    for h in range(Hh):
        out[h * cc:(h + 1) * cc, h] = a[h]
    return out


def chunk_rows(m, p=128):
    R, Cc = m.shape
    n = (R + p - 1) // p
    out = np.zeros((n, p, Cc), m.dtype)
    for i in range(n):
        out[i, :min(p, R - i * p)] = m[i * p:(i + 1) * p]
    return out


def build_program(W, T):
    WS = W * 128
    nc = bass.Bass("TRN2")
    P = {}

    def par(name, shape, dt):
        P[name] = nc.declare_dram_parameter(name, list(shape), dt, isOutput=False)
        return P[name]

    par("xT", [FX, N], DT_F32)
    par("w1bd", [H * FX, F], DT_BF)       # blockdiag of per-head W1
    par("wa1c", [FX, 16], DT_BF)          # W1 @ [bd(as1)|bd(ad1)]
    par("w2c", [4, 128, F], DT_BF)
    par("wa2c", [4, 128, 16], DT_BF)
    par("w3c", [4, 128, F3 + 16], DT_BF)  # [W3 | W3@bd(as3) | W3@bd(ad3)]
    par("b1t", [128, F], DT_BF)
    par("b2t", [128, F], DT_BF)
    par("b3t", [128, C3], DT_F32)
    par("es1", [128, T], DT_I32)
    par("es2", [128, T], DT_I32)
    par("wng", [128, W], DT_I32)
    par("selcat", [128, T * 256], DT_BF)
    OUT = nc.declare_dram_parameter("out", [WS, C3], DT_F32, isOutput=True)

    NT1 = (N + 127) // 128
    with tile.TileContext(nc) as tc:
        with (
            tc.tile_pool(name="const", bufs=1) as cp,
            tc.tile_pool(name="hgp", bufs=3) as hgp,
            tc.tile_pool(name="selp", bufs=3) as selp,
            tc.tile_pool(name="msgp", bufs=3) as msgp,
            tc.tile_pool(name="smp", bufs=4) as smp,
            tc.tile_pool(name="stg", bufs=3) as stg,
            tc.tile_pool(name="pAgg", bufs=2, space="PSUM") as pAgg,
            tc.tile_pool(name="pPrj", bufs=2, space="PSUM") as pPrj,
            tc.tile_pool(name="pDen", bufs=2, space="PSUM") as pDen,
            tc.tile_pool(name="pSm", bufs=2, space="PSUM") as pSm,
            tc.tile_pool(name="dram", bufs=1, space="DRAM") as dr,
        ):
            from concourse.masks import make_identity
            ident = cp.tile([128, 128], DT_F32)
            make_identity(nc, ident[:])
            ident_bf = cp.tile([128, 128], DT_BF)
            nc.vector.tensor_copy(out=ident_bf[:], in_=ident[:])

            t_es1 = cp.tile([128, T], DT_I32)
            nc.sync.dma_start(out=t_es1[:], in_=P["es1"][:])
            t_es2 = cp.tile([128, T], DT_I32)
            nc.sync.dma_start(out=t_es2[:], in_=P["es2"][:])
            t_wng = cp.tile([128, W], DT_I32)
            nc.sync.dma_start(out=t_wng[:], in_=P["wng"][:])

            w1bd = cp.tile([H * FX, F], DT_BF)
            nc.scalar.dma_start(out=w1bd[:], in_=P["w1bd"][:])
            wa1 = cp.tile([FX, 16], DT_BF)
            nc.scalar.dma_start(out=wa1[:], in_=P["wa1c"][:])
            w2 = cp.tile([128, 4, F], DT_BF)
            wa2 = cp.tile([128, 4, 16], DT_BF)
            w3 = cp.tile([128, 4, F3 + 16], DT_BF)
            for ch in range(4):
                nc.scalar.dma_start(out=w2[:, ch, :], in_=P["w2c"][ch])
                nc.scalar.dma_start(out=wa2[:, ch, :], in_=P["wa2c"][ch])
                nc.scalar.dma_start(out=w3[:, ch, :], in_=P["w3c"][ch])
            t_b1 = cp.tile([128, F], DT_BF)
            nc.scalar.dma_start(out=t_b1[:], in_=P["b1t"][:])
            t_b2 = cp.tile([128, F], DT_BF)
            nc.scalar.dma_start(out=t_b2[:], in_=P["b2t"][:])
            t_b3 = cp.tile([128, C3], DT_F32)
            nc.scalar.dma_start(out=t_b3[:], in_=P["b3t"][:])
            hmAll = cp.tile([128, W, C3], DT_F32)

            # ---------------- DRAM internals ----------------
            X1 = dr.tile([N, XR], DT_BF)                # x | als1 | ald1
            exch_h = dr.tile([WS, F + 8], DT_BF)        # h2 | als2 (slot rows)
            Hf2 = dr.tile([NC * WS, F + 8], DT_BF, addr_space="Shared")
            ALD2 = dr.tile([WS, 8], DT_BF)
            exch3 = dr.tile([WS, F3 + 8], DT_F32)       # h3 | als3
            H3f = dr.tile([NC * WS, F3 + 8], DT_F32, addr_space="Shared")
            ALD3 = dr.tile([WS, 8], DT_BF)

            # ------------- layer-1 node-lite phase: build X1 ----------------
            CHT = 50
            NB = 4                      # node tiles per batched DMA write
            xT_sb = None
            for t0 in range(0, NT1, NB):
                nb = min(NB, NT1 - t0)
                xstage = stg.tile([128, NB, XR], DT_BF, tag="x1s")
                rows_last = 128
                for t in range(t0, t0 + nb):
                    rows = min(128, N - t * 128)
                    rows_last = rows
                    if t % CHT == 0:
                        csz = min(CHT * 128, N - t * 128)
                        xT_sb = smp.tile([FX, CHT * 128], DT_BF, tag="xT", bufs=2)
                        nc.gpsimd.dma_start(out=xT_sb[:, :csz],
                                            in_=P["xT"][:, t * 128:t * 128 + csz])
                    off = (t % CHT) * 128
                    lhs = xT_sb[:, off:off + rows]
                    j = t - t0
                    px = pSm.tile([128, FX], DT_BF, space="PSUM", tag="psm")
                    nc.tensor.transpose(px[:rows, :], lhs, ident_bf[:FX, :FX])
                    pa = pSm.tile([128, 16], DT_F32, space="PSUM", tag="psm")
                    nc.tensor.matmul(pa[:rows], lhsT=lhs, rhs=wa1[:], start=True, stop=True)
                    nc.vector.tensor_copy(out=xstage[:rows, j, 0:FX], in_=px[:rows])
                    nc.vector.tensor_copy(out=xstage[:rows, j, FX:], in_=pa[:rows])
                r0 = t0 * 128
                if (nb - 1) * 128 + rows_last == nb * 128:
                    xb = _v(X1[r0:r0 + 128, :], [[128 * XR, nb], [1, XR]])
                    nc.sync.dma_start(out=xb, in_=xstage[:, :nb, :])
                else:
                    for t in range(t0, t0 + nb):
                        rows = min(128, N - t * 128)
                        j = t - t0
                        nc.sync.dma_start(out=X1[t * 128:t * 128 + rows, :],
                                          in_=xstage[:rows, j, :])

            # ---------------- edge phase ----------------
            def edge_phase(layer, Hsrc, es_t):
                lay3 = layer == 3
                # gathered row layout / widths
                RW = XR if layer == 1 else (F3 + 8 if lay3 else F + 8)
                FH = FX if layer == 1 else (F3 if lay3 else F)   # payload width
                CW = FX if layer == 1 else (C3 if lay3 else C)   # per-head width
                AGW = FH * H if layer == 1 else FH               # aggregate width
                fold_den = layer != 2        # exb rides in msg/pout tail
                gdt = DT_F32 if lay3 else DT_BF
                for w in range(W):
                    # source-row gathers, one per tile
                    hg = hgp.tile([128, K_TILES, RW], gdt,
                                  tag="hg1" if layer == 1 else ("hg3" if lay3 else "hg2"))
                    for j in range(K_TILES):
                        nc.gpsimd.indirect_dma_start(
                            out=hg[:, j, :], out_offset=None, in_=Hsrc[:],
                            in_offset=bass.IndirectOffsetOnAxis(
                                ap=es_t[:, w * K_TILES + j:w * K_TILES + j + 1], axis=0))
                    # window al_d [d, 8]
                    adw = smp.tile([128, 8], DT_BF, tag="adw")
                    if layer == 1:
                        nc.gpsimd.indirect_dma_start(
                            out=adw[:], out_offset=None, in_=X1[:],
                            in_offset=bass.IndirectOffsetOnAxis(
                                ap=t_wng[:, w:w + 1], axis=0),
                            element_offset=FX + 8)
                    else:
                        ALDsrc = ALD2 if layer == 2 else ALD3
                        nc.scalar.dma_start(out=adw[:],
                                            in_=ALDsrc[w * 128:(w + 1) * 128, :])
                    # selection matrices for the window (prebuilt in DRAM)
                    sels = selp.tile([128, 16, 128], DT_BF, tag="sels")
                    nc.scalar.dma_start(out=sels[:],
                                        in_=P["selcat"][:, w * 2048:(w + 1) * 2048])
                    pout = pAgg.tile([128, AGW + 8 if fold_den else AGW], DT_F32,
                                     space="PSUM", tag="pout")
                    if not fold_den:
                        pden = pDen.tile([128, 8], DT_F32, space="PSUM", tag="pden")
                    for g0 in range(0, K_TILES, G):
                        gn = G
                        # al_d per edge via selT one-hot matmuls
                        pad_ps = pSm.tile([128, G * 8], DT_F32, space="PSUM", tag="psm")
                        for j in range(gn):
                            nc.tensor.matmul(pad_ps[:, j * 8:(j + 1) * 8],
                                             lhsT=sels[:, 8 + g0 + j, :], rhs=adw[:],
                                             start=True, stop=True)
                        als_b = _v(hg[:, g0, FH:FH + 8], [[RW, gn], [1, 8]])
                        pad_v = _v(pad_ps[:], [[8, gn], [1, 8]])
                        e_t = smp.tile([128, G, 8], DT_F32, tag="e")
                        nc.vector.tensor_tensor(out=e_t[:], in0=als_b, in1=pad_v,
                                                op=ALU.add)
                        ex1 = smp.tile([128, G, 8], DT_F32, tag="ex1")
                        nc.scalar.activation(ex1[:], e_t[:], AF.Exp)
                        ex2 = smp.tile([128, G, 8], DT_F32, tag="ex2")
                        nc.scalar.activation(ex2[:], e_t[:], AF.Exp, scale=NEG_SLOPE)
                        exb = smp.tile([128, G, 8], DT_BF, tag="exb")
                        nc.vector.tensor_tensor(out=exb[:], in0=ex1[:], in1=ex2[:],
                                                op=ALU.max)
                        # msg: payload x ex per head
                        MW = AGW + 8 if fold_den else AGW
                        msg = msgp.tile([128, G, MW], DT_BF, tag="msg")
                        if layer == 1:
                            hg4 = _v(hg[:, g0, 0:FX], [[RW, gn], [0, H], [1, FX]])
                        else:
                            hg4 = _v(hg[:, g0, 0:FH], [[RW, gn], [CW, H], [1, CW]])
                        ex4 = _v(exb[:], [[8, gn], [1, H], [0, CW]])
                        ms4 = _v(msg[:], [[MW, gn], [CW, H], [1, CW]])
                        nc.vector.tensor_tensor(out=ms4, in0=hg4, in1=ex4, op=ALU.mult)
                        if fold_den:
                            nc.vector.tensor_copy(out=msg[:, :, AGW:], in_=exb[:])
                        for j in range(gn):
                            st = (g0 == 0 and j == 0)
                            sp = (g0 + gn == K_TILES and j == gn - 1)
                            nc.tensor.matmul(pout[:], lhsT=sels[:, g0 + j, :],
                                             rhs=msg[:, j, :], start=st, stop=sp)
                            if not fold_den:
                                nc.tensor.matmul(pden[:], lhsT=sels[:, g0 + j, :],
                                                 rhs=exb[:, j, :], start=st, stop=sp)
                    # ---- window close ----
                    den = smp.tile([128, 8], DT_F32, tag="den")
                    nc.vector.tensor_scalar_add(
                        den[:], pout[:, AGW:AGW + 8] if fold_den else pden[:], 1e-16)
                    rec = smp.tile([128, 8], DT_F32, tag="rec")
                    nc.vector.reciprocal(rec[:], den[:])
                    rec_b = _v(rec[:], [[1, H], [0, CW]])
                    po4 = _v(pout[:], [[CW, H], [1, CW]])
                    if lay3:
                        onrm = smp.tile([128, F3], DT_F32, tag="onrm")
                        on4 = _v(onrm[:], [[CW, H], [1, CW]])
                        nc.vector.tensor_tensor(out=on4, in0=po4, in1=rec_b, op=ALU.mult)
                        hm_v = _v(onrm[:], [[1, C3], [C3, H]])
                        hms = smp.tile([128, C3], DT_F32, tag="hms")
                        nc.vector.tensor_reduce(hms[:], hm_v, axis=mybir.AxisListType.X,
                                                op=ALU.add)
                        nc.vector.tensor_scalar_mul(hmAll[:, w, :], hms[:], 0.125)
                        continue
                    if layer == 1:
                        # xw = normalized per-head weighted x sums [d, 8*12]
                        xw = smp.tile([128, H * FX], DT_BF, tag="xw")
                        xw4 = _v(xw[:], [[CW, H], [1, CW]])
                        nc.vector.tensor_tensor(out=xw4, in0=po4, in1=rec_b, op=ALU.mult)
                        pxt = pSm.tile([128, 128], DT_BF, space="PSUM", tag="psm")
                        nc.tensor.transpose(pxt[:H * FX, :], xw[:], ident_bf[:])
                        xwT = smp.tile([H * FX, 128], DT_BF, tag="xwT")
                        nc.vector.tensor_copy(out=xwT[:], in_=pxt[:H * FX, :])
                        ph = pPrj.tile([128, F], DT_F32, space="PSUM", tag="ph")
                        nc.tensor.matmul(ph[:], lhsT=xwT[:], rhs=w1bd[:],
                                         start=True, stop=True)
                        bias = t_b1
                    else:
                        ph = pout          # already h-space [128, 512]
                        bias = t_b2
                        # normalize in place into xn below
                    xn = stg.tile([128, F], DT_BF, tag="xn")
                    if layer == 1:
                        nc.vector.tensor_tensor(out=xn[:], in0=ph[:], in1=bias[:],
                                                op=ALU.add)
                    else:
                        tmp = stg.tile([128, F], DT_BF, tag="tmpn")
                        nc.vector.tensor_tensor(out=_v(tmp[:], [[CW, H], [1, CW]]),
                                                in0=po4, in1=rec_b, op=ALU.mult)
                        nc.vector.tensor_tensor(out=xn[:], in0=tmp[:], in1=bias[:],
                                                op=ALU.add)
                    nc.vector.tensor_scalar_max(xn[:], xn[:], 0.0)
                    xnT = stg.tile([128, 4, 128], DT_BF, tag="xnT")
                    nc.sync.dma_start_transpose(xnT[:], xn[:])
                    if layer == 1:
                        ph2 = pPrj.tile([128, F], DT_F32, space="PSUM", tag="ph")
                        pa2 = pSm.tile([128, 16], DT_F32, space="PSUM", tag="psm")
                        for ch in range(4):
                            nc.tensor.matmul(ph2[:], lhsT=xnT[:, ch, :], rhs=w2[:, ch, :],
                                             start=(ch == 0), stop=(ch == 3))
                            nc.tensor.matmul(pa2[:], lhsT=xnT[:, ch, :], rhs=wa2[:, ch, :],
                                             start=(ch == 0), stop=(ch == 3))
                        h2s = stg.tile([128, F + 8], DT_BF, tag="h2s")
                        nc.vector.tensor_copy(out=h2s[:, :F], in_=ph2[:])
                        nc.vector.tensor_copy(out=h2s[:, F:], in_=pa2[:, 0:8])
                        a2s = stg.tile([128, 8], DT_BF, tag="a2s")
                        nc.vector.tensor_copy(out=a2s[:], in_=pa2[:, 8:16])
                        nc.scalar.dma_start(out=exch_h[w * 128:(w + 1) * 128, :],
                                            in_=h2s[:])
                        nc.scalar.dma_start(out=ALD2[w * 128:(w + 1) * 128, :],
                                            in_=a2s[:])
                    else:
                        ph3 = pPrj.tile([128, F3 + 16], DT_F32, space="PSUM", tag="ph")
                        for ch in range(4):
                            nc.tensor.matmul(ph3[:], lhsT=xnT[:, ch, :], rhs=w3[:, ch, :],
                                             start=(ch == 0), stop=(ch == 3))
                        h3s = stg.tile([128, F3 + 8], DT_F32, tag="h3s")
                        nc.vector.tensor_copy(out=h3s[:], in_=ph3[:, 0:F3 + 8])
                        a3s = stg.tile([128, 8], DT_BF, tag="a3s")
                        nc.vector.tensor_copy(out=a3s[:], in_=ph3[:, F3 + 8:])
                        nc.scalar.dma_start(out=exch3[w * 128:(w + 1) * 128, :],
                                            in_=h3s[:])
                        nc.scalar.dma_start(out=ALD3[w * 128:(w + 1) * 128, :],
                                            in_=a3s[:])

            rg = [list(range(NC))]
            edge_phase(1, X1, t_es1)
            nc.gpsimd.collective_compute("AllGather", ALU.bypass, replica_groups=rg,
                                         ins=[exch_h[:].opt()], outs=[Hf2[:].opt()])
            edge_phase(2, Hf2, t_es2)
            nc.gpsimd.collective_compute("AllGather", ALU.bypass, replica_groups=rg,
                                         ins=[exch3[:].opt()], outs=[H3f[:].opt()])
            edge_phase(3, H3f, t_es2)

            # ---------------- batched log_softmax over all windows ----------
            hmB = cp.tile([128, W, C3], DT_F32)
            b3_b = _v(t_b3[:], [[0, W], [1, C3]])
            nc.vector.tensor_tensor(out=hmB[:], in0=hmAll[:], in1=b3_b, op=ALU.add)
            mx = smp.tile([128, W], DT_F32, tag="mx")
            nc.vector.tensor_reduce(mx[:], hmB[:], axis=mybir.AxisListType.X,
                                    op=ALU.max)
            xc = cp.tile([128, W, C3], DT_F32)
            mx_b = _v(mx[:], [[1, W], [0, C3]])
            nc.vector.tensor_tensor(out=xc[:], in0=hmB[:], in1=mx_b,
                                    op=ALU.subtract)
            e5 = cp.tile([128, W, C3], DT_F32)
            nc.scalar.activation(e5[:], xc[:], AF.Exp)
            s5 = smp.tile([128, W], DT_F32, tag="s5")
            nc.vector.tensor_reduce(s5[:], e5[:], axis=mybir.AxisListType.X,
                                    op=ALU.add)
            lg = smp.tile([128, W], DT_F32, tag="lg")
            nc.scalar.activation(lg[:], s5[:], AF.Ln)
            res = cp.tile([128, W, C3], DT_F32)
            lg_b = _v(lg[:], [[1, W], [0, C3]])
            nc.vector.tensor_tensor(out=res[:], in0=xc[:], in1=lg_b,
                                    op=ALU.subtract)
            out_v = _v(bass.AP(OUT[:].tensor, 0, [[C3, 128]]),
                       [[128 * C3, W], [1, C3]])
            nc.sync.dma_start(out=out_v, in_=res[:])

    _split_drain_waits(nc)
    return nc


_CACHE = {}
_last_in_maps = None


def kernel(**inputs):
    x = np.asarray(inputs["x"], np.float32)
    edge_index = np.asarray(inputs["edge_index"], np.int32)
    es1, es2, wng, selcat, W, T, slot_of = host_prep(edge_index)
    key = (W, T)
    if key not in _CACHE:
        _CACHE[key] = build_program(W, T)
    nc = _CACHE[key]

    W1 = np.asarray(inputs["W1"], np.float32)
    W2 = np.asarray(inputs["W2"], np.float32)
    W3 = np.asarray(inputs["W3"], np.float32)
    wa1 = W1 @ np.concatenate(
        [blockdiag(np.asarray(inputs["as1"])), blockdiag(np.asarray(inputs["ad1"]))], 1)
    wa2 = W2 @ np.concatenate(
        [blockdiag(np.asarray(inputs["as2"])), blockdiag(np.asarray(inputs["ad2"]))], 1)
    wa3 = W3 @ np.concatenate(
        [blockdiag(np.asarray(inputs["as3"])), blockdiag(np.asarray(inputs["ad3"]))], 1)
    w3cat = np.concatenate([W3, wa3], axis=1)

    w1bd = np.zeros((H * FX, F), np.float32)
    for h in range(H):
        w1bd[h * FX:(h + 1) * FX, h * C:(h + 1) * C] = W1[:, h * C:(h + 1) * C]
    com = {
        "xT": np.ascontiguousarray(x.T),
        "w1bd": w1bd.astype(BF),
        "wa1c": wa1.astype(BF),
        "w2c": chunk_rows(W2).astype(BF),
        "wa2c": chunk_rows(wa2).astype(BF),
        "w3c": chunk_rows(w3cat).astype(BF),
        "b1t": np.tile(np.asarray(inputs["b1"], np.float32)[None, :],
                       (128, 1)).astype(BF),
        "b2t": np.tile(np.asarray(inputs["b2"], np.float32)[None, :],
                       (128, 1)).astype(BF),
        "b3t": np.tile(np.asarray(inputs["b3"], np.float32)[None, :], (128, 1)),
    }
    in_maps = []
    for c in range(NC):
        m = dict(com)
        m["es1"] = es1[c]
        m["es2"] = es2[c]
        m["wng"] = wng[c]
        m["selcat"] = selcat[c]
        in_maps.append(m)
    global _last_in_maps
    _last_in_maps = in_maps
    res = run_bass_kernel_spmd(nc, in_maps, list(range(NC)))
    out = np.empty((N, C3), np.float32)
    for c in range(NC):
        out[c * SHARD:(c + 1) * SHARD] = res.results[c]["out"][slot_of[c]]
    return out


# revision 31
# speedup vs baseline: 1.0039x; 1.0039x over previous
"""3-layer GAT (GATConv x3 + log_softmax) on 8 trn2 NeuronCores — v3.

Strategy (dst-node 1-D partition, slot-major exchange):
- Edges on the dst-owner core, sorted by dst, packed in windows of <=128 dst
  nodes / <=K_TILES*128 edges. Per-window DRAM addressing is slot-major
  (window w <-> rows [w*128,(w+1)*128)), identical on every core, so all
  window writes/reads are plain DMAs; the slot->node reorder is host numpy.
- Layer 1 aggregates in x-space: since sum_e alpha_e * (x W1)[s_e] =
  (sum_e alpha_e x[s_e]) W1 per head, the per-edge gather is 56B (x|als|ald
  bf16) instead of 1KB, there is no materialized h1 table, and the per-tile
  aggregation matmul streams 96 cols instead of 512.
- Per-edge source rows come via per-tile indirect DMA ([128,1] offsets is
  the only layout the INDIRECT1D ucode supports). al_d comes per *window*
  (<=128 dst nodes): layer 1 via one tiny indirect gather, layers 2/3 via a
  plain slot-major load; it is expanded per edge with a one-hot matmul
  against a host-prebuilt selT matrix streamed from DRAM (selcat).
- exp(lrelu(x)) = max(exp(x), exp(0.2x)) keeps the scalar engine pinned to
  the Exp table; layer-3 log_softmax is batched over all windows at the end
  (one Exp + one Ln total).
- Segment softmax + scatter via one-hot sel matmuls accumulating in PSUM;
  next-layer features are produced per-window (fused projection) using an
  xbar DMA transpose; h2/h3 exchanges are single AllGathers.
"""
import numpy as np
import ml_dtypes

import concourse.bass as bass
import concourse.mybir as mybir
import concourse.tile as tile
from concourse.bass_utils import run_bass_kernel_spmd

BF = ml_dtypes.bfloat16
N = 50000
NC = 8
SHARD = N // NC            # 6250
H, C = 8, 64
F = H * C                  # 512
C3 = 5
F3 = H * C3                # 40
FX = 12                    # input feature width
XR = 28                    # X1 row: x(12) | als1(8) | ald1(8)
K_TILES = 8
G = 4
NEG_SLOPE = 0.2
DT_BF = mybir.dt.bfloat16
DT_F32 = mybir.dt.float32
DT_I32 = mybir.dt.int32
AF = mybir.ActivationFunctionType
ALU = mybir.AluOpType


def _split_drain_waits(nc, max_waits=1):
    # walrus on this toolchain rejects instructions carrying more than a few
    # sync waits; keep <=max_waits per instruction, move extras onto NoOps
    # inserted just before (same engine -> executes first, semantics kept).
    ctr = 0
    for f in nc.m.functions:
        for blk in f.blocks:
            new_list = []
            for ins in blk.instructions:
                if ins.sync_info and \
                        len(ins.sync_info.on_wait) > max_waits:
                    waits = list(ins.sync_info.on_wait)
                    keep, extra = waits[:max_waits], waits[max_waits:]
                    for w in extra:
                        ctr += 1
                        new_list.append(mybir.InstNoOp(
                            name=f"drainfix-{ctr}", engine=ins.engine,
                            ins=[], outs=[],
                            sync_info=mybir.SyncInfo(on_wait=[w], on_update=[])))
                    ins.sync_info.on_wait = keep
                new_list.append(ins)
            blk.instructions[:] = new_list


def _v(ap, dims):
    """AP over ap's tensor/offset with explicit free [step, count] dims."""
    return bass.AP(ap.tensor, ap.offset, [ap.ap[0]] + dims)


def host_prep(edge_index):
    src = np.concatenate([edge_index[0], np.arange(N, dtype=np.int32)]).astype(np.int64)
    dst = np.concatenate([edge_index[1], np.arange(N, dtype=np.int32)]).astype(np.int64)
    order = np.argsort(dst, kind="stable")
    src, dst = src[order], dst[order]
    cap = K_TILES * 128
    cores = []
    for c in range(NC):
        lo, hi = c * SHARD, (c + 1) * SHARD
        m0 = np.searchsorted(dst, lo, "left")
        m1 = np.searchsorted(dst, hi, "left")
        s_c, d_c = src[m0:m1], dst[m0:m1] - lo
        counts = np.bincount(d_c, minlength=SHARD)
        starts = np.concatenate([[0], np.cumsum(counts)])
        wins = []
        n0 = 0
        while n0 < SHARD:
            n1 = n0
            while n1 < SHARD and (n1 - n0) < 128 and \
                    (starts[n1 + 1] - starts[n0]) <= cap:
                n1 += 1
            if n1 == n0:
                n1 = n0 + 1
            wins.append((n0, n1))
            n0 = n1
        cores.append((s_c, d_c, starts, wins))
    W = max(len(c[3]) for c in cores)
    T = W * K_TILES
    WS = W * 128

    slot_of = np.zeros((NC, SHARD), np.int64)
    for c, (_, _, _, wins) in enumerate(cores):
        for w, (n0, n1) in enumerate(wins):
            slot_of[c, n0:n1] = w * 128 + np.arange(n1 - n0)

    def h2row(s):                        # global node -> row in Hf2 [NC*WS]
        return (s // SHARD) * WS + slot_of[s // SHARD, s % SHARD]

    es1 = np.zeros((NC, T, 128), np.int32)
    es2 = np.zeros((NC, T, 128), np.int32)
    wng = np.zeros((NC, W, 128), np.int32)       # window nodes (global id)
    # selcat: per window 2048 cols: 8 sel tiles [e,d] then 8 selT tiles [d,e]
    selcat = np.zeros((NC, 128, T * 256), BF)
    for c, (s_c, d_c, starts, wins) in enumerate(cores):
        for w, (n0, n1) in enumerate(wins):
            e0, e1 = starts[n0], starts[n1]
            ne = e1 - e0
            t0 = w * K_TILES
            ss = s_c[e0:e1]
            es1[c, t0:t0 + K_TILES].reshape(-1)[:ne] = ss
            es2[c, t0:t0 + K_TILES].reshape(-1)[:ne] = h2row(ss)
            wng[c, w, :n1 - n0] = np.arange(n0, n1) + c * SHARD
            dr = np.full(K_TILES * 128, 999, np.int64)
            dr[:ne] = d_c[e0:e1] - n0
            base = w * 2048
            for j in range(K_TILES):
                drj = dr[j * 128:(j + 1) * 128]
                m = drj < 128
                e_pos = np.nonzero(m)[0]
                d_pos = drj[m]
                blk = np.zeros((128, 128), BF)
                blk[e_pos, d_pos] = 1.0
                selcat[c, :, base + j * 128: base + (j + 1) * 128] = blk
                selcat[c, :, base + 1024 + j * 128: base + 1024 + (j + 1) * 128] = blk.T
    return (np.ascontiguousarray(es1.transpose(0, 2, 1)),
            np.ascontiguousarray(es2.transpose(0, 2, 1)),
            np.ascontiguousarray(wng.transpose(0, 2, 1)),
            selcat, W, T, slot_of)


def blockdiag(a):
    Hh, cc = a.shape
    out = np.zeros((Hh * cc, Hh), np.float32)
    for h in range(Hh):
        out[h * cc:(h + 1) * cc, h] = a[h]
    return out


def chunk_rows(m, p=128):
    R, Cc = m.shape
    n = (R + p - 1) // p
    out = np.zeros((n, p, Cc), m.dtype)
    for i in range(n):
        out[i, :min(p, R - i * p)] = m[i * p:(i + 1) * p]
    return out


def build_program(W, T):
    WS = W * 128
    nc = bass.Bass("TRN2")
    P = {}

    def par(name, shape, dt):
        P[name] = nc.declare_dram_parameter(name, list(shape), dt, isOutput=False)
        return P[name]

    par("xT", [FX, N], DT_F32)
    par("w1bd", [H * FX, F], DT_BF)       # blockdiag of per-head W1
    par("wa1c", [FX, 16], DT_BF)          # W1 @ [bd(as1)|bd(ad1)]
    par("w2c", [4, 128, F], DT_BF)
    par("wa2c", [4, 128, 16], DT_BF)
    par("w3c", [4, 128, F3 + 16], DT_BF)  # [W3 | W3@bd(as3) | W3@bd(ad3)]
    par("b1t", [128, F], DT_BF)
    par("b2t", [128, F], DT_BF)
    par("b3t", [128, C3], DT_F32)
    par("es1", [128, T], DT_I32)
    par("es2", [128, T], DT_I32)
    par("wng", [128, W], DT_I32)
    par("selcat", [128, T * 256], DT_BF)
    OUT = nc.declare_dram_parameter("out", [WS, C3], DT_F32, isOutput=True)

    NT1 = (N + 127) // 128
    with tile.TileContext(nc) as tc:
        with (
            tc.tile_pool(name="const", bufs=1) as cp,
            tc.tile_pool(name="smp", bufs=4) as smp,
            tc.tile_pool(name="stg", bufs=4) as stg,
            tc.tile_pool(name="dram", bufs=1, space="DRAM") as dr,
        ):
            from concourse.masks import make_identity
            ident = cp.tile([128, 128], DT_F32)
            make_identity(nc, ident[:])
            ident_bf = cp.tile([128, 128], DT_BF)
            nc.vector.tensor_copy(out=ident_bf[:], in_=ident[:])

            t_es1 = cp.tile([128, T], DT_I32)
            nc.sync.dma_start(out=t_es1[:], in_=P["es1"][:])
            t_es2 = cp.tile([128, T], DT_I32)
            nc.sync.dma_start(out=t_es2[:], in_=P["es2"][:])
            t_wng = cp.tile([128, W], DT_I32)
            nc.sync.dma_start(out=t_wng[:], in_=P["wng"][:])

            w1bd = cp.tile([H * FX, F], DT_BF)
            nc.scalar.dma_start(out=w1bd[:], in_=P["w1bd"][:])
            wa1 = cp.tile([FX, 16], DT_BF)
            nc.scalar.dma_start(out=wa1[:], in_=P["wa1c"][:])
            w2 = cp.tile([128, 4, F], DT_BF)
            wa2 = cp.tile([128, 4, 16], DT_BF)
            w3 = cp.tile([128, 4, F3 + 16], DT_BF)
            for ch in range(4):
                nc.scalar.dma_start(out=w2[:, ch, :], in_=P["w2c"][ch])
                nc.scalar.dma_start(out=wa2[:, ch, :], in_=P["wa2c"][ch])
                nc.scalar.dma_start(out=w3[:, ch, :], in_=P["w3c"][ch])
            t_b1 = cp.tile([128, F], DT_BF)
            nc.scalar.dma_start(out=t_b1[:], in_=P["b1t"][:])
            t_b2 = cp.tile([128, F], DT_BF)
            nc.scalar.dma_start(out=t_b2[:], in_=P["b2t"][:])
            t_b3 = cp.tile([128, C3], DT_F32)
            nc.scalar.dma_start(out=t_b3[:], in_=P["b3t"][:])
            hmAll = cp.tile([128, W, C3], DT_F32)

            # ---------------- DRAM internals ----------------
            X1 = dr.tile([N, XR], DT_BF)                # x | als1 | ald1
            exch_h = dr.tile([WS, F + 8], DT_BF)        # h2 | als2 (slot rows)
            Hf2 = dr.tile([NC * WS, F + 8], DT_BF, addr_space="Shared")
            ALD2 = dr.tile([WS, 8], DT_BF)
            exch3 = dr.tile([WS, F3 + 8], DT_F32)       # h3 | als3
            H3f = dr.tile([NC * WS, F3 + 8], DT_F32, addr_space="Shared")
            ALD3 = dr.tile([WS, 8], DT_BF)

            # ------------- layer-1 node-lite phase: build X1 ----------------
            CHT = 50
            NB = 4                      # node tiles per batched DMA write
            xT_sb = None
            with tc.tile_pool(name="pNd", bufs=4, space="PSUM") as pNd:
                for t0 in range(0, NT1, NB):
                    nb = min(NB, NT1 - t0)
                    xstage = stg.tile([128, NB, XR], DT_BF, tag="x1s")
                    rows_last = 128
                    for t in range(t0, t0 + nb):
                        rows = min(128, N - t * 128)
                        rows_last = rows
                        if t % CHT == 0:
                            csz = min(CHT * 128, N - t * 128)
                            xT_sb = smp.tile([FX, CHT * 128], DT_BF, tag="xT", bufs=2)
                            nc.gpsimd.dma_start(out=xT_sb[:, :csz],
                                                in_=P["xT"][:, t * 128:t * 128 + csz])
                        off = (t % CHT) * 128
                        lhs = xT_sb[:, off:off + rows]
                        j = t - t0
                        px = pNd.tile([128, FX], DT_BF, space="PSUM", tag="psm")
                        nc.tensor.transpose(px[:rows, :], lhs, ident_bf[:FX, :FX])
                        pa = pNd.tile([128, 16], DT_F32, space="PSUM", tag="psm")
                        nc.tensor.matmul(pa[:rows], lhsT=lhs, rhs=wa1[:],
                                         start=True, stop=True)
                        nc.vector.tensor_copy(out=xstage[:rows, j, 0:FX], in_=px[:rows])
                        nc.vector.tensor_copy(out=xstage[:rows, j, FX:], in_=pa[:rows])
                    r0 = t0 * 128
                    if (nb - 1) * 128 + rows_last == nb * 128:
                        xb = _v(X1[r0:r0 + 128, :], [[128 * XR, nb], [1, XR]])
                        nc.sync.dma_start(out=xb, in_=xstage[:, :nb, :])
                    else:
                        for t in range(t0, t0 + nb):
                            rows = min(128, N - t * 128)
                            j = t - t0
                            nc.sync.dma_start(out=X1[t * 128:t * 128 + rows, :],
                                              in_=xstage[:rows, j, :])

            # ---------------- edge phase ----------------
            def edge_phase(layer, Hsrc, es_t, pools):
                hgp, selp, msgp, pAgg, pPrj, pDen, pSm = pools
                lay3 = layer == 3
                # gathered row layout / widths
                RW = XR if layer == 1 else (F3 + 8 if lay3 else F + 8)
                FH = FX if layer == 1 else (F3 if lay3 else F)   # payload width
                CW = FX if layer == 1 else (C3 if lay3 else C)   # per-head width
                AGW = FH * H if layer == 1 else FH               # aggregate width
                fold_den = layer != 2        # exb rides in msg/pout tail
                gdt = DT_F32 if lay3 else DT_BF
                for w in range(W):
                    # source-row gathers, one per tile
                    hg = hgp.tile([128, K_TILES, RW], gdt,
                                  tag="hg1" if layer == 1 else ("hg3" if lay3 else "hg2"))
                    for j in range(K_TILES):
                        nc.gpsimd.indirect_dma_start(
                            out=hg[:, j, :], out_offset=None, in_=Hsrc[:],
                            in_offset=bass.IndirectOffsetOnAxis(
                                ap=es_t[:, w * K_TILES + j:w * K_TILES + j + 1], axis=0))
                    # window al_d [d, 8]
                    adw = smp.tile([128, 8], DT_BF, tag="adw")
                    if layer == 1:
                        nc.gpsimd.indirect_dma_start(
                            out=adw[:], out_offset=None, in_=X1[:],
                            in_offset=bass.IndirectOffsetOnAxis(
                                ap=t_wng[:, w:w + 1], axis=0),
                            element_offset=FX + 8)
                    else:
                        ALDsrc = ALD2 if layer == 2 else ALD3
                        nc.scalar.dma_start(out=adw[:],
                                            in_=ALDsrc[w * 128:(w + 1) * 128, :])
                    # selection matrices for the window (prebuilt in DRAM)
                    sels = selp.tile([128, 16, 128], DT_BF, tag="sels")
                    nc.sync.dma_start(out=sels[:],
                                      in_=P["selcat"][:, w * 2048:(w + 1) * 2048])
                    pout = pAgg.tile([128, AGW + 8 if fold_den else AGW], DT_F32,
                                     space="PSUM", tag="pout")
                    if not fold_den:
                        pden = pDen.tile([128, 8], DT_F32, space="PSUM", tag="pden")
                    for g0 in range(0, K_TILES, G):
                        gn = G
                        # al_d per edge via selT one-hot matmuls
                        pad_ps = pSm.tile([128, G * 8], DT_F32, space="PSUM", tag="psm")
                        for j in range(gn):
                            nc.tensor.matmul(pad_ps[:, j * 8:(j + 1) * 8],
                                             lhsT=sels[:, 8 + g0 + j, :], rhs=adw[:],
                                             start=True, stop=True)
                        als_b = _v(hg[:, g0, FH:FH + 8], [[RW, gn], [1, 8]])
                        pad_v = _v(pad_ps[:], [[8, gn], [1, 8]])
                        e_t = smp.tile([128, G, 8], DT_F32, tag="e")
                        nc.vector.tensor_tensor(out=e_t[:], in0=als_b, in1=pad_v,
                                                op=ALU.add)
                        ex1 = smp.tile([128, G, 8], DT_F32, tag="ex1")
                        nc.scalar.activation(ex1[:], e_t[:], AF.Exp)
                        ex2 = smp.tile([128, G, 8], DT_F32, tag="ex2")
                        nc.scalar.activation(ex2[:], e_t[:], AF.Exp, scale=NEG_SLOPE)
                        exb = smp.tile([128, G, 8], DT_BF, tag="exb")
                        nc.vector.tensor_tensor(out=exb[:], in0=ex1[:], in1=ex2[:],
                                                op=ALU.max)
                        # msg: payload x ex per head
                        MW = AGW + 8 if fold_den else AGW
                        msg = msgp.tile([128, G, MW], DT_BF, tag="msg")
                        if layer == 1:
                            hg4 = _v(hg[:, g0, 0:FX], [[RW, gn], [0, H], [1, FX]])
                        else:
                            hg4 = _v(hg[:, g0, 0:FH], [[RW, gn], [CW, H], [1, CW]])
                        ex4 = _v(exb[:], [[8, gn], [1, H], [0, CW]])
                        ms4 = _v(msg[:], [[MW, gn], [CW, H], [1, CW]])
                        nc.vector.tensor_tensor(out=ms4, in0=hg4, in1=ex4, op=ALU.mult)
                        if fold_den:
                            nc.vector.tensor_copy(out=msg[:, :, AGW:], in_=exb[:])
                        for j in range(gn):
                            st = (g0 == 0 and j == 0)
                            sp = (g0 + gn == K_TILES and j == gn - 1)
                            nc.tensor.matmul(pout[:], lhsT=sels[:, g0 + j, :],
                                             rhs=msg[:, j, :], start=st, stop=sp)
                            if not fold_den:
                                nc.tensor.matmul(pden[:], lhsT=sels[:, g0 + j, :],
                                                 rhs=exb[:, j, :], start=st, stop=sp)
                    # ---- window close ----
                    den = smp.tile([128, 8], DT_F32, tag="den")
                    nc.vector.tensor_scalar_add(
                        den[:], pout[:, AGW:AGW + 8] if fold_den else pden[:], 1e-16)
                    rec = smp.tile([128, 8], DT_F32, tag="rec")
                    nc.vector.reciprocal(rec[:], den[:])
                    rec_b = _v(rec[:], [[1, H], [0, CW]])
                    po4 = _v(pout[:], [[CW, H], [1, CW]])
                    if lay3:
                        onrm = smp.tile([128, F3], DT_F32, tag="onrm")
                        on4 = _v(onrm[:], [[CW, H], [1, CW]])
                        nc.vector.tensor_tensor(out=on4, in0=po4, in1=rec_b, op=ALU.mult)
                        hm_v = _v(onrm[:], [[1, C3], [C3, H]])
                        hms = smp.tile([128, C3], DT_F32, tag="hms")
                        nc.vector.tensor_reduce(hms[:], hm_v, axis=mybir.AxisListType.X,
                                                op=ALU.add)
                        nc.vector.tensor_scalar_mul(hmAll[:, w, :], hms[:], 0.125)
                        continue
                    if layer == 1:
                        # xw = normalized per-head weighted x sums [d, 8*12]
                        xw = smp.tile([128, H * FX], DT_BF, tag="xw")
                        xw4 = _v(xw[:], [[CW, H], [1, CW]])
                        nc.vector.tensor_tensor(out=xw4, in0=po4, in1=rec_b, op=ALU.mult)
                        pxt = pSm.tile([128, 128], DT_BF, space="PSUM", tag="psm")
                        nc.tensor.transpose(pxt[:H * FX, :], xw[:], ident_bf[:])
                        xwT = smp.tile([H * FX, 128], DT_BF, tag="xwT")
                        nc.vector.tensor_copy(out=xwT[:], in_=pxt[:H * FX, :])
                        ph = pPrj.tile([128, F], DT_F32, space="PSUM", tag="ph")
                        nc.tensor.matmul(ph[:], lhsT=xwT[:], rhs=w1bd[:],
                                         start=True, stop=True)
                        bias = t_b1
                    else:
                        ph = pout          # already h-space [128, 512]
                        bias = t_b2
                        # normalize in place into xn below
                    xn = stg.tile([128, F], DT_BF, tag="xn")
                    if layer == 1:
                        nc.vector.tensor_tensor(out=xn[:], in0=ph[:], in1=bias[:],
                                                op=ALU.add)
                    else:
                        tmp = stg.tile([128, F], DT_BF, tag="tmpn")
                        nc.vector.tensor_tensor(out=_v(tmp[:], [[CW, H], [1, CW]]),
                                                in0=po4, in1=rec_b, op=ALU.mult)
                        nc.vector.tensor_tensor(out=xn[:], in0=tmp[:], in1=bias[:],
                                                op=ALU.add)
                    nc.vector.tensor_scalar_max(xn[:], xn[:], 0.0)
                    xnT = stg.tile([128, 4, 128], DT_BF, tag="xnT")
                    nc.sync.dma_start_transpose(xnT[:], xn[:])
                    if layer == 1:
                        ph2 = pPrj.tile([128, F], DT_F32, space="PSUM", tag="ph")
                        pa2 = pSm.tile([128, 16], DT_F32, space="PSUM", tag="psm")
                        for ch in range(4):
                            nc.tensor.matmul(ph2[:], lhsT=xnT[:, ch, :], rhs=w2[:, ch, :],
                                             start=(ch == 0), stop=(ch == 3))
                            nc.tensor.matmul(pa2[:], lhsT=xnT[:, ch, :], rhs=wa2[:, ch, :],
                                             start=(ch == 0), stop=(ch == 3))
                        h2s = stg.tile([128, F + 8], DT_BF, tag="h2s")
                        nc.vector.tensor_copy(out=h2s[:, :F], in_=ph2[:])
                        nc.vector.tensor_copy(out=h2s[:, F:], in_=pa2[:, 0:8])
                        a2s = stg.tile([128, 8], DT_BF, tag="a2s")
                        nc.vector.tensor_copy(out=a2s[:], in_=pa2[:, 8:16])
                        nc.scalar.dma_start(out=exch_h[w * 128:(w + 1) * 128, :],
                                            in_=h2s[:])
                        nc.scalar.dma_start(out=ALD2[w * 128:(w + 1) * 128, :],
                                            in_=a2s[:])
                    else:
                        ph3 = pPrj.tile([128, F3 + 16], DT_F32, space="PSUM", tag="ph")
                        for ch in range(4):
                            nc.tensor.matmul(ph3[:], lhsT=xnT[:, ch, :], rhs=w3[:, ch, :],
                                             start=(ch == 0), stop=(ch == 3))
                        h3s = stg.tile([128, F3 + 8], DT_F32, tag="h3s")
                        nc.vector.tensor_copy(out=h3s[:], in_=ph3[:, 0:F3 + 8])
                        a3s = stg.tile([128, 8], DT_BF, tag="a3s")
                        nc.vector.tensor_copy(out=a3s[:], in_=ph3[:, F3 + 8:])
                        nc.scalar.dma_start(out=exch3[w * 128:(w + 1) * 128, :],
                                            in_=h3s[:])
                        nc.scalar.dma_start(out=ALD3[w * 128:(w + 1) * 128, :],
                                            in_=a3s[:])

            rg = [list(range(NC))]
            with (
                tc.tile_pool(name="hgp1", bufs=6) as h1p,
                tc.tile_pool(name="selp1", bufs=4) as s1p,
                tc.tile_pool(name="msgp1", bufs=4) as m1p,
                tc.tile_pool(name="pAgg1", bufs=3, space="PSUM") as pA1,
                tc.tile_pool(name="pPrj1", bufs=3, space="PSUM") as pP1,
                tc.tile_pool(name="pSm1", bufs=2, space="PSUM") as pS1,
            ):
                edge_phase(1, X1, t_es1, (h1p, s1p, m1p, pA1, pP1, None, pS1))
            nc.gpsimd.collective_compute("AllGather", ALU.bypass, replica_groups=rg,
                                         ins=[exch_h[:].opt()], outs=[Hf2[:].opt()])
            with (
                tc.tile_pool(name="hgp2", bufs=4) as h2p,
                tc.tile_pool(name="selp2", bufs=4) as s2p,
                tc.tile_pool(name="msgp2", bufs=4) as m2p,
                tc.tile_pool(name="pAgg2", bufs=2, space="PSUM") as pA2,
                tc.tile_pool(name="pPrj2", bufs=2, space="PSUM") as pP2,
                tc.tile_pool(name="pDen2", bufs=2, space="PSUM") as pD2,
                tc.tile_pool(name="pSm2", bufs=2, space="PSUM") as pS2,
            ):
                edge_phase(2, Hf2, t_es2, (h2p, s2p, m2p, pA2, pP2, pD2, pS2))
            nc.gpsimd.collective_compute("AllGather", ALU.bypass, replica_groups=rg,
                                         ins=[exch3[:].opt()], outs=[H3f[:].opt()])
            with (
                tc.tile_pool(name="hgp3", bufs=6) as h3p,
                tc.tile_pool(name="selp3", bufs=4) as s3p,
                tc.tile_pool(name="msgp3", bufs=4) as m3p,
                tc.tile_pool(name="pAgg3", bufs=4, space="PSUM") as pA3,
                tc.tile_pool(name="pSm3", bufs=2, space="PSUM") as pS3,
            ):
                edge_phase(3, H3f, t_es2, (h3p, s3p, m3p, pA3, None, None, pS3))

            # ---------------- batched log_softmax over all windows ----------
            hmB = cp.tile([128, W, C3], DT_F32)
            b3_b = _v(t_b3[:], [[0, W], [1, C3]])
            nc.vector.tensor_tensor(out=hmB[:], in0=hmAll[:], in1=b3_b, op=ALU.add)
            mx = smp.tile([128, W], DT_F32, tag="mx")
            nc.vector.tensor_reduce(mx[:], hmB[:], axis=mybir.AxisListType.X,
                                    op=ALU.max)
            xc = cp.tile([128, W, C3], DT_F32)
            mx_b = _v(mx[:], [[1, W], [0, C3]])
            nc.vector.tensor_tensor(out=xc[:], in0=hmB[:], in1=mx_b,
                                    op=ALU.subtract)
            e5 = cp.tile([128, W, C3], DT_F32)
            nc.scalar.activation(e5[:], xc[:], AF.Exp)
            s5 = smp.tile([128, W], DT_F32, tag="s5")
            nc.vector.tensor_reduce(s5[:], e5[:], axis=mybir.AxisListType.X,
                                    op=ALU.add)
            lg = smp.tile([128, W], DT_F32, tag="lg")
            nc.scalar.activation(lg[:], s5[:], AF.Ln)
            res = cp.tile([128, W, C3], DT_F32)
            lg_b = _v(lg[:], [[1, W], [0, C3]])
            nc.vector.tensor_tensor(out=res[:], in0=xc[:], in1=lg_b,
                                    op=ALU.subtract)
            out_v = _v(bass.AP(OUT[:].tensor, 0, [[C3, 128]]),
                       [[128 * C3, W], [1, C3]])
            nc.sync.dma_start(out=out_v, in_=res[:])

    _split_drain_waits(nc)
    return nc


_CACHE = {}
_last_in_maps = None


def kernel(**inputs):
    x = np.asarray(inputs["x"], np.float32)
    edge_index = np.asarray(inputs["edge_index"], np.int32)
    es1, es2, wng, selcat, W, T, slot_of = host_prep(edge_index)
    key = (W, T)
    if key not in _CACHE:
        _CACHE[key] = build_program(W, T)
    nc = _CACHE[key]

    W1 = np.asarray(inputs["W1"], np.float32)
    W2 = np.asarray(inputs["W2"], np.float32)
    W3 = np.asarray(inputs["W3"], np.float32)
    wa1 = W1 @ np.concatenate(
        [blockdiag(np.asarray(inputs["as1"])), blockdiag(np.asarray(inputs["ad1"]))], 1)
    wa2 = W2 @ np.concatenate(
        [blockdiag(np.asarray(inputs["as2"])), blockdiag(np.asarray(inputs["ad2"]))], 1)
    wa3 = W3 @ np.concatenate(
        [blockdiag(np.asarray(inputs["as3"])), blockdiag(np.asarray(inputs["ad3"]))], 1)
    w3cat = np.concatenate([W3, wa3], axis=1)

    w1bd = np.zeros((H * FX, F), np.float32)
    for h in range(H):
        w1bd[h * FX:(h + 1) * FX, h * C:(h + 1) * C] = W1[:, h * C:(h + 1) * C]
    com = {
        "xT": np.ascontiguousarray(x.T),
        "w1bd": w1bd.astype(BF),
        "wa1c": wa1.astype(BF),
        "w2c": chunk_rows(W2).astype(BF),
        "wa2c": chunk_rows(wa2).astype(BF),
        "w3c": chunk_rows(w3cat).astype(BF),
        "b1t": np.tile(np.asarray(inputs["b1"], np.float32)[None, :],
                       (128, 1)).astype(BF),
        "b2t": np.tile(np.asarray(inputs["b2"], np.float32)[None, :],
                       (128, 1)).astype(BF),
        "b3t": np.tile(np.asarray(inputs["b3"], np.float32)[None, :], (128, 1)),
    }
    in_maps = []
    for c in range(NC):
        m = dict(com)
        m["es1"] = es1[c]
        m["es2"] = es2[c]
        m["wng"] = wng[c]
        m["selcat"] = selcat[c]
        in_maps.append(m)
    global _last_in_maps
    _last_in_maps = in_maps
    res = run_bass_kernel_spmd(nc, in_maps, list(range(NC)))
    out = np.empty((N, C3), np.float32)
    for c in range(NC):
        out[c * SHARD:(c + 1) * SHARD] = res.results[c]["out"][slot_of[c]]
    return out


# revision 32
# speedup vs baseline: 1.2196x; 1.2149x over previous
"""3-layer GAT (GATConv x3 + log_softmax) on 8 trn2 NeuronCores — v3.

Strategy (dst-node 1-D partition, slot-major exchange):
- Edges on the dst-owner core, sorted by dst, packed in windows of <=128 dst
  nodes / <=K_TILES*128 edges. Per-window DRAM addressing is slot-major
  (window w <-> rows [w*128,(w+1)*128)), identical on every core, so all
  window writes/reads are plain DMAs; the slot->node reorder is host numpy.
- Layer 1 aggregates in x-space: since sum_e alpha_e * (x W1)[s_e] =
  (sum_e alpha_e x[s_e]) W1 per head, the per-edge gather is 56B (x|als|ald
  bf16) instead of 1KB, there is no materialized h1 table, and the per-tile
  aggregation matmul streams 96 cols instead of 512.
- Per-edge source rows come via per-tile indirect DMA ([128,1] offsets is
  the only layout the INDIRECT1D ucode supports). al_d comes per *window*
  (<=128 dst nodes): layer 1 via one tiny indirect gather, layers 2/3 via a
  plain slot-major load; it is expanded per edge with a one-hot matmul
  against a host-prebuilt selT matrix streamed from DRAM (selcat).
- exp(lrelu(x)) = max(exp(x), exp(0.2x)) keeps the scalar engine pinned to
  the Exp table; layer-3 log_softmax is batched over all windows at the end
  (one Exp + one Ln total).
- Segment softmax + scatter via one-hot sel matmuls accumulating in PSUM;
  next-layer features are produced per-window (fused projection) using an
  xbar DMA transpose; h2/h3 exchanges are single AllGathers.
"""
import numpy as np
import ml_dtypes

import concourse.bass as bass
import concourse.mybir as mybir
import concourse.tile as tile
from concourse.bass_utils import run_bass_kernel_spmd

BF = ml_dtypes.bfloat16
N = 50000
NC = 8
SHARD = N // NC            # 6250
H, C = 8, 64
F = H * C                  # 512
C3 = 5
F3 = H * C3                # 40
FX = 12                    # input feature width
XR = 28                    # X1 row: x(12) | als1(8) | ald1(8)
K_TILES = 8
G = 4
NEG_SLOPE = 0.2
DT_BF = mybir.dt.bfloat16
DT_F32 = mybir.dt.float32
DT_I32 = mybir.dt.int32
AF = mybir.ActivationFunctionType
ALU = mybir.AluOpType


def _split_drain_waits(nc, max_waits=1):
    # walrus on this toolchain rejects instructions carrying more than a few
    # sync waits; keep <=max_waits per instruction, move extras onto NoOps
    # inserted just before (same engine -> executes first, semantics kept).
    ctr = 0
    for f in nc.m.functions:
        for blk in f.blocks:
            new_list = []
            for ins in blk.instructions:
                if ins.sync_info and \
                        len(ins.sync_info.on_wait) > max_waits:
                    waits = list(ins.sync_info.on_wait)
                    keep, extra = waits[:max_waits], waits[max_waits:]
                    for w in extra:
                        ctr += 1
                        new_list.append(mybir.InstNoOp(
                            name=f"drainfix-{ctr}", engine=ins.engine,
                            ins=[], outs=[],
                            sync_info=mybir.SyncInfo(on_wait=[w], on_update=[])))
                    ins.sync_info.on_wait = keep
                new_list.append(ins)
            blk.instructions[:] = new_list


def _v(ap, dims):
    """AP over ap's tensor/offset with explicit free [step, count] dims."""
    return bass.AP(ap.tensor, ap.offset, [ap.ap[0]] + dims)


def host_prep(edge_index):
    src = np.concatenate([edge_index[0], np.arange(N, dtype=np.int32)]).astype(np.int64)
    dst = np.concatenate([edge_index[1], np.arange(N, dtype=np.int32)]).astype(np.int64)
    order = np.argsort(dst, kind="stable")
    src, dst = src[order], dst[order]
    cap = K_TILES * 128
    cores = []
    for c in range(NC):
        lo, hi = c * SHARD, (c + 1) * SHARD
        m0 = np.searchsorted(dst, lo, "left")
        m1 = np.searchsorted(dst, hi, "left")
        s_c, d_c = src[m0:m1], dst[m0:m1] - lo
        counts = np.bincount(d_c, minlength=SHARD)
        starts = np.concatenate([[0], np.cumsum(counts)])
        wins = []
        n0 = 0
        while n0 < SHARD:
            n1 = n0
            while n1 < SHARD and (n1 - n0) < 128 and \
                    (starts[n1 + 1] - starts[n0]) <= cap:
                n1 += 1
            if n1 == n0:
                n1 = n0 + 1
            wins.append((n0, n1))
            n0 = n1
        cores.append((s_c, d_c, starts, wins))
    W = max(len(c[3]) for c in cores)
    T = W * K_TILES
    WS = W * 128

    slot_of = np.zeros((NC, SHARD), np.int64)
    for c, (_, _, _, wins) in enumerate(cores):
        for w, (n0, n1) in enumerate(wins):
            slot_of[c, n0:n1] = w * 128 + np.arange(n1 - n0)

    def h2row(s):                        # global node -> row in Hf2 [NC*WS]
        return (s // SHARD) * WS + slot_of[s // SHARD, s % SHARD]

    es1 = np.zeros((NC, T, 128), np.int32)
    es2 = np.zeros((NC, T, 128), np.int32)
    wng = np.zeros((NC, W, 128), np.int32)       # window nodes (global id)
    # selcat: per window 2048 cols: 8 sel tiles [e,d] then 8 selT tiles [d,e]
    selcat = np.zeros((NC, 128, T * 256), BF)
    for c, (s_c, d_c, starts, wins) in enumerate(cores):
        for w, (n0, n1) in enumerate(wins):
            e0, e1 = starts[n0], starts[n1]
            ne = e1 - e0
            t0 = w * K_TILES
            ss = s_c[e0:e1]
            es1[c, t0:t0 + K_TILES].reshape(-1)[:ne] = ss
            es2[c, t0:t0 + K_TILES].reshape(-1)[:ne] = h2row(ss)
            wng[c, w, :n1 - n0] = np.arange(n0, n1) + c * SHARD
            dr = np.full(K_TILES * 128, 999, np.int64)
            dr[:ne] = d_c[e0:e1] - n0
            base = w * 2048
            for j in range(K_TILES):
                drj = dr[j * 128:(j + 1) * 128]
                m = drj < 128
                e_pos = np.nonzero(m)[0]
                d_pos = drj[m]
                blk = np.zeros((128, 128), BF)
                blk[e_pos, d_pos] = 1.0
                selcat[c, :, base + j * 128: base + (j + 1) * 128] = blk
                selcat[c, :, base + 1024 + j * 128: base + 1024 + (j + 1) * 128] = blk.T
    return (np.ascontiguousarray(es1.transpose(0, 2, 1)),
            np.ascontiguousarray(es2.transpose(0, 2, 1)),
            np.ascontiguousarray(wng.transpose(0, 2, 1)),
            selcat, W, T, slot_of)


def blockdiag(a):
    Hh, cc = a.shape
    out = np.zeros((Hh * cc, Hh), np.float32)
    for h in range(Hh):
        out[h * cc:(h + 1) * cc, h] = a[h]
    return out


def chunk_rows(m, p=128):
    R, Cc = m.shape
    n = (R + p - 1) // p
    out = np.zeros((n, p, Cc), m.dtype)
    for i in range(n):
        out[i, :min(p, R - i * p)] = m[i * p:(i + 1) * p]
    return out


def build_program(W, T):
    WS = W * 128
    nc = bass.Bass("TRN2")
    P = {}

    def par(name, shape, dt):
        P[name] = nc.declare_dram_parameter(name, list(shape), dt, isOutput=False)
        return P[name]

    par("xT", [FX, N], DT_F32)
    par("w1bd", [H * FX, F], DT_BF)       # blockdiag of per-head W1
    par("wa1c", [FX, 16], DT_BF)          # W1 @ [bd(as1)|bd(ad1)]
    par("w2c", [4, 128, F], DT_BF)
    par("wa2c", [4, 128, 16], DT_BF)
    par("w3c", [4, 128, F3 + 16], DT_BF)  # [W3 | W3@bd(as3) | W3@bd(ad3)]
    par("b1t", [128, F], DT_BF)
    par("b2t", [128, F], DT_BF)
    par("b3t", [128, C3], DT_F32)
    par("es1", [128, T], DT_I32)
    par("es2", [128, T], DT_I32)
    par("wng", [128, W], DT_I32)
    par("selcat", [128, T * 256], DT_BF)
    OUT = nc.declare_dram_parameter("out", [WS, C3], DT_F32, isOutput=True)

    NT1 = (N + 127) // 128
    with tile.TileContext(nc) as tc:
        with (
            tc.tile_pool(name="const", bufs=1) as cp,
            tc.tile_pool(name="smp", bufs=4) as smp,
            tc.tile_pool(name="stg", bufs=4) as stg,
            tc.tile_pool(name="dram", bufs=1, space="DRAM") as dr,
        ):
            from concourse.masks import make_identity
            ident = cp.tile([128, 128], DT_F32)
            make_identity(nc, ident[:])
            ident_bf = cp.tile([128, 128], DT_BF)
            nc.vector.tensor_copy(out=ident_bf[:], in_=ident[:])

            t_es1 = cp.tile([128, T], DT_I32)
            nc.sync.dma_start(out=t_es1[:], in_=P["es1"][:])
            t_es2 = cp.tile([128, T], DT_I32)
            nc.sync.dma_start(out=t_es2[:], in_=P["es2"][:])
            t_wng = cp.tile([128, W], DT_I32)
            nc.sync.dma_start(out=t_wng[:], in_=P["wng"][:])

            w1bd = cp.tile([H * FX, F], DT_BF)
            nc.scalar.dma_start(out=w1bd[:], in_=P["w1bd"][:])
            wa1 = cp.tile([FX, 16], DT_BF)
            nc.scalar.dma_start(out=wa1[:], in_=P["wa1c"][:])
            w2 = cp.tile([128, 4, F], DT_BF)
            wa2 = cp.tile([128, 4, 16], DT_BF)
            w3 = cp.tile([128, 4, F3 + 16], DT_BF)
            for ch in range(4):
                nc.scalar.dma_start(out=w2[:, ch, :], in_=P["w2c"][ch])
                nc.scalar.dma_start(out=wa2[:, ch, :], in_=P["wa2c"][ch])
                nc.scalar.dma_start(out=w3[:, ch, :], in_=P["w3c"][ch])
            t_b1 = cp.tile([128, F], DT_BF)
            nc.scalar.dma_start(out=t_b1[:], in_=P["b1t"][:])
            t_b2 = cp.tile([128, F], DT_BF)
            nc.scalar.dma_start(out=t_b2[:], in_=P["b2t"][:])
            t_b3 = cp.tile([128, C3], DT_F32)
            nc.scalar.dma_start(out=t_b3[:], in_=P["b3t"][:])
            hmAll = cp.tile([128, W, C3], DT_F32)

            # ---------------- DRAM internals ----------------
            X1 = dr.tile([N, XR], DT_BF)                # x | als1 | ald1
            exch_h = dr.tile([WS, F + 8], DT_BF)        # h2 | als2 (slot rows)
            Hf2 = dr.tile([NC * WS, F + 8], DT_BF, addr_space="Shared")
            ALD2 = dr.tile([WS, 8], DT_BF)
            exch3 = dr.tile([WS, F3 + 8], DT_F32)       # h3 | als3
            H3f = dr.tile([NC * WS, F3 + 8], DT_F32, addr_space="Shared")
            ALD3 = dr.tile([WS, 8], DT_BF)

            # ------------- layer-1 node-lite phase: build X1 ----------------
            CHT = 50
            NB = 4                      # node tiles per batched DMA write
            xT_sb = None
            with tc.tile_pool(name="pNd", bufs=4, space="PSUM") as pNd:
                for t0 in range(0, NT1, NB):
                    nb = min(NB, NT1 - t0)
                    xstage = stg.tile([128, NB, XR], DT_BF, tag="x1s")
                    rows_last = 128
                    for t in range(t0, t0 + nb):
                        rows = min(128, N - t * 128)
                        rows_last = rows
                        if t % CHT == 0:
                            csz = min(CHT * 128, N - t * 128)
                            xT_sb = smp.tile([FX, CHT * 128], DT_BF, tag="xT", bufs=2)
                            nc.gpsimd.dma_start(out=xT_sb[:, :csz],
                                                in_=P["xT"][:, t * 128:t * 128 + csz])
                        off = (t % CHT) * 128
                        lhs = xT_sb[:, off:off + rows]
                        j = t - t0
                        px = pNd.tile([128, FX], DT_BF, space="PSUM", tag="psm")
                        nc.tensor.transpose(px[:rows, :], lhs, ident_bf[:FX, :FX])
                        pa = pNd.tile([128, 16], DT_F32, space="PSUM", tag="psm")
                        nc.tensor.matmul(pa[:rows], lhsT=lhs, rhs=wa1[:],
                                         start=True, stop=True)
                        nc.vector.tensor_copy(out=xstage[:rows, j, 0:FX], in_=px[:rows])
                        nc.vector.tensor_copy(out=xstage[:rows, j, FX:], in_=pa[:rows])
                    r0 = t0 * 128
                    if (nb - 1) * 128 + rows_last == nb * 128:
                        xb = _v(X1[r0:r0 + 128, :], [[128 * XR, nb], [1, XR]])
                        nc.sync.dma_start(out=xb, in_=xstage[:, :nb, :])
                    else:
                        for t in range(t0, t0 + nb):
                            rows = min(128, N - t * 128)
                            j = t - t0
                            nc.sync.dma_start(out=X1[t * 128:t * 128 + rows, :],
                                              in_=xstage[:rows, j, :])

            # ---------------- edge phase ----------------
            def edge_phase(layer, Hsrc, es_t, pools):
                hgp, selp, msgp, pAgg, pPrj, pDen, pSm = pools
                lay3 = layer == 3
                # gathered row layout / widths
                RW = XR if layer == 1 else (F3 + 8 if lay3 else F + 8)
                FH = FX if layer == 1 else (F3 if lay3 else F)   # payload width
                CW = FX if layer == 1 else (C3 if lay3 else C)   # per-head width
                AGW = FH * H if layer == 1 else FH               # aggregate width
                fold_den = layer != 2        # exb rides in msg/pout tail
                gdt = DT_F32 if lay3 else DT_BF
                for w in range(W):
                    # source-row gathers, one per tile
                    hg = hgp.tile([128, K_TILES, RW], gdt,
                                  tag="hg1" if layer == 1 else ("hg3" if lay3 else "hg2"))
                    for j in range(K_TILES):
                        nc.gpsimd.indirect_dma_start(
                            out=hg[:, j, :], out_offset=None, in_=Hsrc[:],
                            in_offset=bass.IndirectOffsetOnAxis(
                                ap=es_t[:, w * K_TILES + j:w * K_TILES + j + 1], axis=0))
                    # window al_d [d, 8]
                    adw = smp.tile([128, 8], DT_BF, tag="adw")
                    if layer == 1:
                        nc.gpsimd.indirect_dma_start(
                            out=adw[:], out_offset=None, in_=X1[:],
                            in_offset=bass.IndirectOffsetOnAxis(
                                ap=t_wng[:, w:w + 1], axis=0),
                            element_offset=FX + 8)
                    else:
                        ALDsrc = ALD2 if layer == 2 else ALD3
                        nc.scalar.dma_start(out=adw[:],
                                            in_=ALDsrc[w * 128:(w + 1) * 128, :])
                    # selection matrices for the window (prebuilt in DRAM)
                    sels = selp.tile([128, 16, 128], DT_BF, tag="sels")
                    nc.sync.dma_start(out=sels[:],
                                      in_=P["selcat"][:, w * 2048:(w + 1) * 2048])
                    pout = pAgg.tile([128, AGW + 8 if fold_den else AGW], DT_F32,
                                     space="PSUM", tag="pout")
                    if not fold_den:
                        pden = pDen.tile([128, 8], DT_F32, space="PSUM", tag="pden")
                    for g0 in range(0, K_TILES, G):
                        gn = G
                        # al_d per edge via selT one-hot matmuls
                        pad_ps = pSm.tile([128, G * 8], DT_F32, space="PSUM", tag="psm")
                        for j in range(gn):
                            nc.tensor.matmul(pad_ps[:, j * 8:(j + 1) * 8],
                                             lhsT=sels[:, 8 + g0 + j, :], rhs=adw[:],
                                             start=True, stop=True)
                        als_b = _v(hg[:, g0, FH:FH + 8], [[RW, gn], [1, 8]])
                        pad_v = _v(pad_ps[:], [[8, gn], [1, 8]])
                        e_t = smp.tile([128, G, 8], DT_F32, tag="e")
                        nc.vector.tensor_tensor(out=e_t[:], in0=als_b, in1=pad_v,
                                                op=ALU.add)
                        ex1 = smp.tile([128, G, 8], DT_F32, tag="ex1")
                        nc.scalar.activation(ex1[:], e_t[:], AF.Exp)
                        ex2 = smp.tile([128, G, 8], DT_F32, tag="ex2")
                        nc.scalar.activation(ex2[:], e_t[:], AF.Exp, scale=NEG_SLOPE)
                        exb = smp.tile([128, G, 8], DT_BF, tag="exb")
                        nc.vector.tensor_tensor(out=exb[:], in0=ex1[:], in1=ex2[:],
                                                op=ALU.max)
                        # msg: payload x ex per head
                        MW = AGW + 8 if fold_den else AGW
                        msg = msgp.tile([128, G, MW], DT_BF, tag="msg")
                        if layer == 1:
                            hg4 = _v(hg[:, g0, 0:FX], [[RW, gn], [0, H], [1, FX]])
                        else:
                            hg4 = _v(hg[:, g0, 0:FH], [[RW, gn], [CW, H], [1, CW]])
                        ex4 = _v(exb[:], [[8, gn], [1, H], [0, CW]])
                        ms4 = _v(msg[:], [[MW, gn], [CW, H], [1, CW]])
                        nc.vector.tensor_tensor(out=ms4, in0=hg4, in1=ex4, op=ALU.mult)
                        if fold_den:
                            nc.vector.tensor_copy(out=msg[:, :, AGW:], in_=exb[:])
                        for j in range(gn):
                            st = (g0 == 0 and j == 0)
                            sp = (g0 + gn == K_TILES and j == gn - 1)
                            nc.tensor.matmul(pout[:], lhsT=sels[:, g0 + j, :],
                                             rhs=msg[:, j, :], start=st, stop=sp)
                            if not fold_den:
                                nc.tensor.matmul(pden[:], lhsT=sels[:, g0 + j, :],
                                                 rhs=exb[:, j, :], start=st, stop=sp)
                    # ---- window close ----
                    den = smp.tile([128, 8], DT_F32, tag="den")
                    nc.vector.tensor_scalar_add(
                        den[:], pout[:, AGW:AGW + 8] if fold_den else pden[:], 1e-16)
                    rec = smp.tile([128, 8], DT_F32, tag="rec")
                    nc.vector.reciprocal(rec[:], den[:])
                    rec_b = _v(rec[:], [[1, H], [0, CW]])
                    po4 = _v(pout[:], [[CW, H], [1, CW]])
                    if lay3:
                        onrm = smp.tile([128, F3], DT_F32, tag="onrm")
                        on4 = _v(onrm[:], [[CW, H], [1, CW]])
                        nc.vector.tensor_tensor(out=on4, in0=po4, in1=rec_b, op=ALU.mult)
                        hm_v = _v(onrm[:], [[1, C3], [C3, H]])
                        hms = smp.tile([128, C3], DT_F32, tag="hms")
                        nc.vector.tensor_reduce(hms[:], hm_v, axis=mybir.AxisListType.X,
                                                op=ALU.add)
                        nc.vector.tensor_scalar_mul(hmAll[:, w, :], hms[:], 0.125)
                        continue
                    if layer == 1:
                        # xw = normalized per-head weighted x sums [d, 8*12]
                        xw = smp.tile([128, H * FX], DT_BF, tag="xw")
                        xw4 = _v(xw[:], [[CW, H], [1, CW]])
                        nc.vector.tensor_tensor(out=xw4, in0=po4, in1=rec_b, op=ALU.mult)
                        pxt = pSm.tile([128, 128], DT_BF, space="PSUM", tag="psm")
                        nc.tensor.transpose(pxt[:H * FX, :], xw[:], ident_bf[:])
                        xwT = smp.tile([H * FX, 128], DT_BF, tag="xwT")
                        nc.vector.tensor_copy(out=xwT[:], in_=pxt[:H * FX, :])
                        ph = pPrj.tile([128, F], DT_F32, space="PSUM", tag="ph")
                        nc.tensor.matmul(ph[:], lhsT=xwT[:], rhs=w1bd[:],
                                         start=True, stop=True)
                        bias = t_b1
                    else:
                        ph = pout          # already h-space [128, 512]
                        bias = t_b2
                        # normalize in place into xn below
                    xn = stg.tile([128, F], DT_BF, tag="xn")
                    if layer == 1:
                        nc.vector.tensor_tensor(out=xn[:], in0=ph[:], in1=bias[:],
                                                op=ALU.add)
                    else:
                        tmp = stg.tile([128, F], DT_BF, tag="tmpn")
                        nc.vector.tensor_tensor(out=_v(tmp[:], [[CW, H], [1, CW]]),
                                                in0=po4, in1=rec_b, op=ALU.mult)
                        nc.vector.tensor_tensor(out=xn[:], in0=tmp[:], in1=bias[:],
                                                op=ALU.add)
                    nc.vector.tensor_scalar_max(xn[:], xn[:], 0.0)
                    xnT = stg.tile([128, 4, 128], DT_BF, tag="xnT")
                    for ch in range(4):
                        ptp = pSm.tile([128, 128], DT_BF, space="PSUM", tag="psm")
                        nc.tensor.transpose(ptp[:], xn[:, ch * 128:(ch + 1) * 128],
                                            ident_bf[:])
                        nc.vector.tensor_copy(out=xnT[:, ch, :], in_=ptp[:])
                    if layer == 1:
                        ph2 = pPrj.tile([128, F], DT_F32, space="PSUM", tag="ph")
                        pa2 = pSm.tile([128, 16], DT_F32, space="PSUM", tag="psm")
                        for ch in range(4):
                            nc.tensor.matmul(ph2[:], lhsT=xnT[:, ch, :], rhs=w2[:, ch, :],
                                             start=(ch == 0), stop=(ch == 3))
                            nc.tensor.matmul(pa2[:], lhsT=xnT[:, ch, :], rhs=wa2[:, ch, :],
                                             start=(ch == 0), stop=(ch == 3))
                        h2s = stg.tile([128, F + 8], DT_BF, tag="h2s")
                        nc.vector.tensor_copy(out=h2s[:, :F], in_=ph2[:])
                        nc.vector.tensor_copy(out=h2s[:, F:], in_=pa2[:, 0:8])
                        a2s = stg.tile([128, 8], DT_BF, tag="a2s")
                        nc.vector.tensor_copy(out=a2s[:], in_=pa2[:, 8:16])
                        nc.scalar.dma_start(out=exch_h[w * 128:(w + 1) * 128, :],
                                            in_=h2s[:])
                        nc.scalar.dma_start(out=ALD2[w * 128:(w + 1) * 128, :],
                                            in_=a2s[:])
                    else:
                        ph3 = pPrj.tile([128, F3 + 16], DT_F32, space="PSUM", tag="ph")
                        for ch in range(4):
                            nc.tensor.matmul(ph3[:], lhsT=xnT[:, ch, :], rhs=w3[:, ch, :],
                                             start=(ch == 0), stop=(ch == 3))
                        h3s = stg.tile([128, F3 + 8], DT_F32, tag="h3s")
                        nc.vector.tensor_copy(out=h3s[:], in_=ph3[:, 0:F3 + 8])
                        a3s = stg.tile([128, 8], DT_BF, tag="a3s")
                        nc.vector.tensor_copy(out=a3s[:], in_=ph3[:, F3 + 8:])
                        nc.scalar.dma_start(out=exch3[w * 128:(w + 1) * 128, :],
                                            in_=h3s[:])
                        nc.scalar.dma_start(out=ALD3[w * 128:(w + 1) * 128, :],
                                            in_=a3s[:])

            rg = [list(range(NC))]
            with (
                tc.tile_pool(name="hgp1", bufs=6) as h1p,
                tc.tile_pool(name="selp1", bufs=4) as s1p,
                tc.tile_pool(name="msgp1", bufs=4) as m1p,
                tc.tile_pool(name="pAgg1", bufs=3, space="PSUM") as pA1,
                tc.tile_pool(name="pPrj1", bufs=3, space="PSUM") as pP1,
                tc.tile_pool(name="pSm1", bufs=2, space="PSUM") as pS1,
            ):
                edge_phase(1, X1, t_es1, (h1p, s1p, m1p, pA1, pP1, None, pS1))
            nc.gpsimd.collective_compute("AllGather", ALU.bypass, replica_groups=rg,
                                         ins=[exch_h[:].opt()], outs=[Hf2[:].opt()])
            with (
                tc.tile_pool(name="hgp2", bufs=4) as h2p,
                tc.tile_pool(name="selp2", bufs=4) as s2p,
                tc.tile_pool(name="msgp2", bufs=4) as m2p,
                tc.tile_pool(name="pAgg2", bufs=2, space="PSUM") as pA2,
                tc.tile_pool(name="pPrj2", bufs=2, space="PSUM") as pP2,
                tc.tile_pool(name="pDen2", bufs=2, space="PSUM") as pD2,
                tc.tile_pool(name="pSm2", bufs=2, space="PSUM") as pS2,
            ):
                edge_phase(2, Hf2, t_es2, (h2p, s2p, m2p, pA2, pP2, pD2, pS2))
            nc.gpsimd.collective_compute("AllGather", ALU.bypass, replica_groups=rg,
                                         ins=[exch3[:].opt()], outs=[H3f[:].opt()])
            with (
                tc.tile_pool(name="hgp3", bufs=6) as h3p,
                tc.tile_pool(name="selp3", bufs=4) as s3p,
                tc.tile_pool(name="msgp3", bufs=4) as m3p,
                tc.tile_pool(name="pAgg3", bufs=4, space="PSUM") as pA3,
                tc.tile_pool(name="pSm3", bufs=2, space="PSUM") as pS3,
            ):
                edge_phase(3, H3f, t_es2, (h3p, s3p, m3p, pA3, None, None, pS3))

            # ---------------- batched log_softmax over all windows ----------
            hmB = cp.tile([128, W, C3], DT_F32)
            b3_b = _v(t_b3[:], [[0, W], [1, C3]])
            nc.vector.tensor_tensor(out=hmB[:], in0=hmAll[:], in1=b3_b, op=ALU.add)
            mx = smp.tile([128, W], DT_F32, tag="mx")
            nc.vector.tensor_reduce(mx[:], hmB[:], axis=mybir.AxisListType.X,
                                    op=ALU.max)
            xc = cp.tile([128, W, C3], DT_F32)
            mx_b = _v(mx[:], [[1, W], [0, C3]])
            nc.vector.tensor_tensor(out=xc[:], in0=hmB[:], in1=mx_b,
                                    op=ALU.subtract)
            e5 = cp.tile([128, W, C3], DT_F32)
            nc.scalar.activation(e5[:], xc[:], AF.Exp)
            s5 = smp.tile([128, W], DT_F32, tag="s5")
            nc.vector.tensor_reduce(s5[:], e5[:], axis=mybir.AxisListType.X,
                                    op=ALU.add)
            lg = smp.tile([128, W], DT_F32, tag="lg")
            nc.scalar.activation(lg[:], s5[:], AF.Ln)
            res = cp.tile([128, W, C3], DT_F32)
            lg_b = _v(lg[:], [[1, W], [0, C3]])
            nc.vector.tensor_tensor(out=res[:], in0=xc[:], in1=lg_b,
                                    op=ALU.subtract)
            out_v = _v(bass.AP(OUT[:].tensor, 0, [[C3, 128]]),
                       [[128 * C3, W], [1, C3]])
            nc.sync.dma_start(out=out_v, in_=res[:])

    _split_drain_waits(nc)
    return nc


_CACHE = {}
_last_in_maps = None


def kernel(**inputs):
    x = np.asarray(inputs["x"], np.float32)
    edge_index = np.asarray(inputs["edge_index"], np.int32)
    es1, es2, wng, selcat, W, T, slot_of = host_prep(edge_index)
    key = (W, T)
    if key not in _CACHE:
        _CACHE[key] = build_program(W, T)
    nc = _CACHE[key]

    W1 = np.asarray(inputs["W1"], np.float32)
    W2 = np.asarray(inputs["W2"], np.float32)
    W3 = np.asarray(inputs["W3"], np.float32)
    wa1 = W1 @ np.concatenate(
        [blockdiag(np.asarray(inputs["as1"])), blockdiag(np.asarray(inputs["ad1"]))], 1)
    wa2 = W2 @ np.concatenate(
        [blockdiag(np.asarray(inputs["as2"])), blockdiag(np.asarray(inputs["ad2"]))], 1)
    wa3 = W3 @ np.concatenate(
        [blockdiag(np.asarray(inputs["as3"])), blockdiag(np.asarray(inputs["ad3"]))], 1)
    w3cat = np.concatenate([W3, wa3], axis=1)

    w1bd = np.zeros((H * FX, F), np.float32)
    for h in range(H):
        w1bd[h * FX:(h + 1) * FX, h * C:(h + 1) * C] = W1[:, h * C:(h + 1) * C]
    com = {
        "xT": np.ascontiguousarray(x.T),
        "w1bd": w1bd.astype(BF),
        "wa1c": wa1.astype(BF),
        "w2c": chunk_rows(W2).astype(BF),
        "wa2c": chunk_rows(wa2).astype(BF),
        "w3c": chunk_rows(w3cat).astype(BF),
        "b1t": np.tile(np.asarray(inputs["b1"], np.float32)[None, :],
                       (128, 1)).astype(BF),
        "b2t": np.tile(np.asarray(inputs["b2"], np.float32)[None, :],
                       (128, 1)).astype(BF),
        "b3t": np.tile(np.asarray(inputs["b3"], np.float32)[None, :], (128, 1)),
    }
    in_maps = []
    for c in range(NC):
        m = dict(com)
        m["es1"] = es1[c]
        m["es2"] = es2[c]
        m["wng"] = wng[c]
        m["selcat"] = selcat[c]
        in_maps.append(m)
    global _last_in_maps
    _last_in_maps = in_maps
    res = run_bass_kernel_spmd(nc, in_maps, list(range(NC)))
    out = np.empty((N, C3), np.float32)
    for c in range(NC):
        out[c * SHARD:(c + 1) * SHARD] = res.results[c]["out"][slot_of[c]]
    return out
